# revision 3
# baseline (speedup 1.0000x reference)
"""DVAE encoder (batched DAG GRU message passing) on 8 trn2 NeuronCores.

Pure data-parallel over batch (256 graphs/core). Feature-major GRU compute
(features on partitions, batch on free dim) with weight-stationary bf16
matmuls at N=256. v2 structure:
  - Step 0 is host-side: hv0/msg0 are 8-entry embedding tables (h starts
    at zero), so the kernel starts at step 1 with hT1 and msb[u=0] DMA'd.
  - Bn (= w_ih_n @ one-hot x + b_ih_n) is a host-side gather, DMA'd per
    step (double-buffered).
  - wgm3 (kc=3 gate/mapper chunk) is 97% static across steps: two
    resident ping-pong tiles hold the static rows; only the per-step
    row-125 fold (vertex-id columns + gate bias, 2KB) is DMA'd.
  - Diagonal adjacency mask tiles are host-built; their DMA is staged
    one tile per step (vn=v+2 fetched at step v) so the initial DMA
    burst never starves the per-step bn/row tensors (HAM stays warm).
  - GRU matmul order r, c, z lets the pointwise chain start as soon as
    ps_c lands; the full aggregation prefix for step v+1 fills the PE
    gap during the pointwise chain (no dummy-transpose filler).
  - gate/mapper matmuls run batch-major (hv chunks stationary, weights
    moving) so the message write is a single DVE multiply into msb.
  - Blend uses z' = 1-z: hv = z'*n + z*h, with z*h on GpSimd off the
    critical chain.
"""

import numpy as np

B, MAX_N, NVT, HS, NZ = 2048, 16, 8, 501, 56
NC_CORES = 8
BL = B // NC_CORES     # 256 per core
NBT = BL // 128        # 2 batch tiles

_CACHE = {}


def _tri(vn):
    return vn * (vn - 1) // 2


def _build_nc():
    import concourse.mybir as mybir
    import concourse.tile as tile
    from concourse import bacc

    F32 = mybir.dt.float32
    BF = mybir.dt.bfloat16
    AF = mybir.ActivationFunctionType

    nc = bacc.Bacc("TRN2", target_bir_lowering=False, debug=False,
                   num_devices=NC_CORES)

    d_wa = nc.dram_tensor("wa", [128, 4 * 1024], BF, kind="ExternalInput").ap()
    d_wc = nc.dram_tensor("wc", [128, 4 * 512], BF, kind="ExternalInput").ap()
    d_wgm = nc.dram_tensor("wgm", [128, 3 * 1024], BF, kind="ExternalInput").ap()
    d_w3s = nc.dram_tensor("w3s", [128, 1024], BF, kind="ExternalInput").ap()
    d_w3r = nc.dram_tensor("w3r", [16, 1024], BF, kind="ExternalInput").ap()
    d_bnf = nc.dram_tensor("bnf", [128, 16 * 1024], BF, kind="ExternalInput").ap()
    d_dmf = [nc.dram_tensor(f"dmf{bt}", [128, 120 * 128], BF,
                            kind="ExternalInput").ap() for bt in range(NBT)]
    d_xh = nc.dram_tensor("xh", [128, 16 * NBT * 9], BF, kind="ExternalInput").ap()
    d_ht1 = nc.dram_tensor("ht1", [128, 1024], BF, kind="ExternalInput").ap()
    d_ms0 = nc.dram_tensor("ms0", [128, NBT * 512], BF, kind="ExternalInput").ap()
    d_wf = nc.dram_tensor("wf", [128, 4 * 112], BF, kind="ExternalInput").ap()
    d_fcb = nc.dram_tensor("fcb", [128, 1], F32, kind="ExternalInput").ap()
    d_id = nc.dram_tensor("ident", [128, 128], BF, kind="ExternalInput").ap()
    d_y = nc.dram_tensor("y", [112, BL], F32, kind="ExternalOutput").ap()

    with tile.TileContext(nc) as tc:
        with tc.tile_pool(name="wts", bufs=1) as wts, \
             tc.tile_pool(name="dbuf", bufs=2) as dbuf, \
             tc.tile_pool(name="state", bufs=1) as state, \
             tc.tile_pool(name="work", bufs=2) as work, \
             tc.tile_pool(name="hbmp", bufs=2) as hbmp, \
             tc.tile_pool(name="psA", bufs=2, space="PSUM") as psA, \
             tc.tile_pool(name="pagg", bufs=2, space="PSUM") as paggp, \
             tc.tile_pool(name="psT", bufs=2, space="PSUM") as psT:

            ident = wts.tile([128, 128], BF, tag="ident", name="ident")
            nc.sync.dma_start(out=ident[:], in_=d_id[:])

            # PE warmup during the input DMA: keeps the HAM clock gate
            # open so step-1 matmuls run at full rate.
            pwarm = psA.tile([128, 4, 256], F32, tag="psA", name="pwarm")
            for i in range(48):
                nc.tensor.matmul(pwarm[:, 0, 0:128], ident[:], ident[:],
                                 start=True, stop=True)

            # --- initial DMAs, critical-first ---
            hT = work.tile([128, 4, 256], BF, tag="hT", name="hT1")
            nc.sync.dma_start(out=hT[:], in_=d_ht1[:])
            wa = wts.tile([128, 4 * 1024], BF, tag="wa", name="wa")
            for kc in range(4):
                nc.sync.dma_start(out=wa[:, kc * 1024:(kc + 1) * 1024],
                                  in_=d_wa[:, kc * 1024:(kc + 1) * 1024])
            wc = wts.tile([128, 4 * 512], BF, tag="wc", name="wc")
            for kc in range(4):
                nc.sync.dma_start(out=wc[:, kc * 512:(kc + 1) * 512],
                                  in_=d_wc[:, kc * 512:(kc + 1) * 512])

            # per-step Bn, double-buffered ([128, 4, 256] view of a flat
            # [128, 1024] dram slice)
            def fetch_bn(v):
                t = dbuf.tile([128, 4, 256], BF, tag="bn", name=f"bn{v}")
                nc.sync.dma_start(out=t[:], in_=d_bnf[:, v * 1024:(v + 1) * 1024])
                return t

            bn_cur = fetch_bn(1)

            xh = wts.tile([128, 16 * NBT * 9], BF, tag="xh", name="xh")
            nc.sync.dma_start(out=xh[:], in_=d_xh[:])

            # messages, batch-major: [128b, u(16) * bt(2) * 512f];
            # slot u=0 holds the host-computed msg0
            msb = state.tile([128, MAX_N * NBT * 512], BF, tag="msb",
                             name="msb")
            nc.sync.dma_start(out=msb[:, 0:NBT * 512], in_=d_ms0[:])

            wgm = wts.tile([128, 3 * 1024], BF, tag="wgm", name="wgm")
            for kc in range(3):
                nc.sync.dma_start(out=wgm[:, kc * 1024:(kc + 1) * 1024],
                                  in_=d_wgm[:, kc * 1024:(kc + 1) * 1024])
            # kc=3 gate/mapper chunk: static rows resident in two
            # ping-pong tiles; row 125 (per-step fold) DMA'd per step.
            t3 = [wts.tile([128, 4, 256], BF, tag=f"w3_{i}", name=f"w3_{i}")
                  for i in range(2)]
            for i in range(2):
                nc.sync.dma_start(out=t3[i][:], in_=d_w3s[:])
            nc.sync.dma_start(out=t3[1][125:126, :, :], in_=d_w3r[1:2, :])
            nc.sync.dma_start(out=t3[0][125:126, :, :], in_=d_w3r[2:3, :])

            # per-step diagonal mask tiles (vn = 2..15), staged: vn is
            # fetched one step before first use to keep DMA queues clear
            dmt = [[None] * MAX_N for _ in range(NBT)]

            def dm_fetch(vn):
                for bt in range(NBT):
                    t = wts.tile([128, vn * 128], BF, tag=f"dm{bt}_{vn}",
                                 name=f"dm{bt}_{vn}")
                    dmt[bt][vn] = t
                    nc.sync.dma_start(
                        out=t[:],
                        in_=d_dmf[bt][:, _tri(vn) * 128:(_tri(vn) + vn) * 128])

            dm_fetch(2)
            dm_fetch(3)
            wf = wts.tile([128, 4 * 112], BF, tag="wf", name="wf")
            nc.sync.dma_start(out=wf[:], in_=d_wf[:])
            fcb = wts.tile([128, 1], F32, tag="fcb", name="fcb")
            nc.sync.dma_start(out=fcb[:], in_=d_fcb[:])

            def bs(t, bt):
                return t[:, :, bt * 128:(bt + 1) * 128]

            for v in range(1, MAX_N):
                vn = v + 1
                if v < MAX_N - 1:
                    bn_nxt = fetch_bn(v + 1)
                if 4 <= v + 2 <= 15:
                    dm_fetch(v + 2)

                # ---- GRU matmuls: r first, then c (pointwise needs it
                # second), then z ----
                ps_r = psA.tile([128, 4, 256], F32, tag="psA", name="ps_r")
                for mt in range(4):
                    for kc in range(4):
                        nc.tensor.matmul(
                            ps_r[:, mt, :],
                            wa[:, kc * 1024 + mt * 128:kc * 1024 + mt * 128 + 128],
                            hT[:, kc, :], start=(kc == 0), stop=(kc == 3))
                ps_c = psA.tile([128, 4, 256], F32, tag="psA", name="ps_c")
                for mt in range(4):
                    for kc in range(4):
                        nc.tensor.matmul(
                            ps_c[:, mt, :],
                            wc[:, kc * 512 + mt * 128:kc * 512 + mt * 128 + 128],
                            hT[:, kc, :], start=(kc == 0), stop=(kc == 3))
                ps_z = psA.tile([128, 4, 256], F32, tag="psA", name="ps_z")
                for mt in range(4):
                    for kc in range(4):
                        nc.tensor.matmul(
                            ps_z[:, mt, :],
                            wa[:, kc * 1024 + 512 + mt * 128:
                               kc * 1024 + 512 + mt * 128 + 128],
                            hT[:, kc, :], start=(kc == 0), stop=(kc == 3))

                # ---- aggregation prefix for step v+1 fills the
                # pointwise gap ----
                pags = []
                nbefore = min(v, 8)
                if vn < MAX_N:
                    for bt in range(NBT):
                        pag = paggp.tile([128, 512], F32, tag="pagg",
                                         name="pag")
                        pags.append(pag)
                        for u in range(nbefore):
                            nc.tensor.matmul(
                                pag[:],
                                dmt[bt][vn][:, u * 128:u * 128 + 128],
                                msb[:, (u * NBT + bt) * 512:
                                    (u * NBT + bt) * 512 + 512],
                                start=(u == 0), stop=False)

                # ---- GRU pointwise ----
                r = work.tile([128, 4, 256], BF, tag="r", name="r")
                z = work.tile([128, 4, 256], BF, tag="z", name="z")
                z2 = work.tile([128, 4, 256], BF, tag="z2", name="z2")
                u_t = work.tile([128, 4, 256], BF, tag="u_t", name="u_t")
                t_t = work.tile([128, 4, 256], BF, tag="t_t", name="t_t")
                n_t = work.tile([128, 4, 256], BF, tag="n_t", name="n_t")
                q_t = work.tile([128, 4, 256], BF, tag="q_t", name="q_t")
                p_t = work.tile([128, 4, 256], BF, tag="p_t", name="p_t")
                hv = work.tile([128, 4, 256], BF, tag="hv", name="hv")

                for bt in range(NBT):
                    nc.scalar.activation(bs(r, bt), bs(ps_r, bt), AF.Sigmoid)
                for bt in range(NBT):
                    nc.vector.tensor_mul(bs(u_t, bt), bs(r, bt), bs(ps_c, bt))
                for bt in range(NBT):
                    nc.vector.tensor_add(bs(t_t, bt), bs(u_t, bt),
                                         bs(bn_cur, bt))
                for bt in range(NBT):
                    nc.scalar.activation(bs(n_t, bt), bs(t_t, bt), AF.Tanh)
                for bt in range(NBT):
                    nc.scalar.activation(bs(z, bt), bs(ps_z, bt), AF.Sigmoid)
                for bt in range(NBT):
                    nc.gpsimd.tensor_mul(bs(q_t, bt), bs(z, bt), bs(hT, bt))
                for bt in range(NBT):
                    nc.vector.tensor_scalar(
                        bs(z2, bt), bs(z, bt), -1.0, 1.0,
                        op0=mybir.AluOpType.mult, op1=mybir.AluOpType.add)
                for bt in range(NBT):
                    nc.vector.tensor_mul(bs(p_t, bt), bs(z2, bt), bs(n_t, bt))
                for bt in range(NBT):
                    nc.vector.tensor_add(bs(hv, bt), bs(p_t, bt), bs(q_t, bt))

                if v < MAX_N - 1:
                    hT_next = work.tile([128, 4, 256], BF, tag="hT",
                                        name="hT")
                    hbm = []
                    for bt in range(NBT):
                        # ---- gate/mapper matmuls, batch-major ----
                        gmp = psA.tile([128, 4, 256], F32, tag="psA",
                                       name="gmp")

                        def gm_half(half):
                            for kc in range(3):
                                nc.tensor.matmul(
                                    gmp[:, half * 2:half * 2 + 2, :],
                                    hv[:, kc, bt * 128:bt * 128 + 128],
                                    wgm[:, kc * 1024 + half * 512:
                                        kc * 1024 + half * 512 + 512],
                                    start=(kc == 0), stop=False)
                            nc.tensor.matmul(
                                gmp[:, half * 2:half * 2 + 2, :],
                                hv[:, 3, bt * 128:bt * 128 + 128],
                                t3[v & 1][:, half * 2:half * 2 + 2, :],
                                start=False, stop=True)

                        gm_half(0)
                        g = work.tile([128, 512], BF, tag=f"g{bt}",
                                      name=f"g{bt}")
                        nc.scalar.activation(g[:], gmp[:, 0:2, :], AF.Sigmoid)
                        gm_half(1)
                        # remainder of the aggregation prefix
                        for u in range(nbefore, v):
                            nc.tensor.matmul(
                                pags[bt][:],
                                dmt[bt][vn][:, u * 128:u * 128 + 128],
                                msb[:, (u * NBT + bt) * 512:
                                    (u * NBT + bt) * 512 + 512],
                                start=False, stop=False)
                        moff = (v * NBT + bt) * 512
                        nc.vector.tensor_mul(msb[:, moff:moff + 512],
                                             gmp[:, 2:4, :], g[:])
                        # ---- final aggregation term (this step's message) ----
                        nc.tensor.matmul(
                            pags[bt][:],
                            dmt[bt][vn][:, v * 128:v * 128 + 128],
                            msb[:, moff:moff + 512],
                            start=False, stop=True)
                        hb = hbmp.tile([128, 512], BF, tag=f"hbm{bt}",
                                       name=f"hbm{bt}")
                        hbm.append(hb)
                        ptp = psT.tile([128, 4, 128], BF, tag="ptp",
                                       name="ptp")
                        nc.scalar.copy(hb[:, 0:256], pags[bt][:, 0:256])
                        for kc in range(2):
                            nc.tensor.transpose(
                                ptp[:, kc, :],
                                hb[:, kc * 128:kc * 128 + 128], ident[:])
                        nc.scalar.copy(hb[:, 256:512], pags[bt][:, 256:512])
                        nc.vector.tensor_copy(
                            hb[:, 501:510],
                            xh[:, (vn * NBT + bt) * 9:(vn * NBT + bt) * 9 + 9])
                        for kc in range(2, 4):
                            nc.tensor.transpose(
                                ptp[:, kc, :],
                                hb[:, kc * 128:kc * 128 + 128], ident[:])
                        nc.vector.tensor_copy(
                            hT_next[:, :, bt * 128:bt * 128 + 128], ptp[:])

                    # row-125 fold for step v+2 into the tile this step
                    # just finished reading
                    if v + 2 <= MAX_N - 2:
                        nc.sync.dma_start(out=t3[v & 1][125:126, :, :],
                                          in_=d_w3r[v + 2:v + 3, :])
                    hT = hT_next
                    bn_cur = bn_nxt
                else:
                    # ---- final FC, per-bt to overlap the pointwise ----
                    pf = psA.tile([128, 4, 256], F32, tag="psA", name="pf")
                    out_sb = work.tile([128, 256], F32, tag="out_sb",
                                       name="out_sb")
                    for bt in range(NBT):
                        for kc in range(4):
                            nc.tensor.matmul(
                                pf[:112, 0, bt * 128:bt * 128 + 128],
                                wf[:, kc * 112:kc * 112 + 112],
                                hv[:, kc, bt * 128:bt * 128 + 128],
                                start=(kc == 0), stop=(kc == 3))
                        nc.scalar.activation(
                            out_sb[:112, bt * 128:bt * 128 + 128],
                            pf[:112, 0, bt * 128:bt * 128 + 128],
                            AF.Identity, bias=fcb[:112, :])
                    nc.sync.dma_start(out=d_y[:], in_=out_sb[:112, :])

    nc.compile()
    return nc


def _sigmoid(x):
    return 1.0 / (1.0 + np.exp(-x))


def _prep_static(w_ih, w_hh, b_ih, b_hh, gate_w, gate_b, map_w,
                 fc1_w, fc1_b, fc2_w, fc2_b):
    import ml_dtypes
    f32 = np.float32
    bf16 = ml_dtypes.bfloat16
    bias = (b_ih + b_hh).astype(f32)

    WA = np.zeros((512, 1024), f32)
    WA[0:501, 0:501] = w_hh[0:501].T
    WA[501:509, 0:501] = w_ih[0:501].T
    WA[509, 0:501] = bias[0:501]
    WA[0:501, 512:1013] = w_hh[501:1002].T
    WA[501:509, 512:1013] = w_ih[501:1002].T
    WA[509, 512:1013] = bias[501:1002]
    # z-padding output 509: sigmoid(30) == 1 exactly in bf16, so
    # hv[509] = z*hT = const-1 propagates for the gate-bias fold
    WA[509, 1021] = 30.0
    WC = np.zeros((512, 512), f32)
    WC[0:501, 0:501] = w_hh[1002:1503].T
    WC[509, 0:501] = b_hh[1002:1503]
    WF = np.zeros((512, 112), f32)
    WF[0:501, 0:56] = fc1_w.T
    WF[0:501, 56:112] = fc2_w.T

    def ktile_flat(W, cols):
        return np.ascontiguousarray(
            W.reshape(4, 128, cols).transpose(1, 0, 2).reshape(128, 4 * cols)
        ).astype(bf16)

    # g/m moving weights: W^T layouts [K(features), N(outputs)]
    WGT = np.zeros((512, 512), f32)
    WGT[0:501, 0:501] = gate_w[:, 0:501].T
    WMT = np.zeros((512, 512), f32)
    WMT[0:501, 0:501] = map_w[:, 0:501].T
    wgm = np.zeros((128, 3 * 1024), f32)
    for kc in range(3):
        wgm[:, kc * 1024:kc * 1024 + 512] = WGT[kc * 128:(kc + 1) * 128]
        wgm[:, kc * 1024 + 512:(kc + 1) * 1024] = WMT[kc * 128:(kc + 1) * 128]
    # kc=3 chunk: static rows 0:117; per-step row 125 (id-col fold)
    w3s = np.zeros((128, 1024), f32)
    w3s[0:117, 0:512] = WGT[384:501]
    w3s[0:117, 512:1024] = WMT[384:501]
    w3r = np.zeros((16, 1024), f32)
    for v in range(16):
        w3r[v, 0:501] = gate_b + gate_w[:, HS + v]
        w3r[v, 512:512 + 501] = map_w[:, HS + v]

    # step-0 embedding tables: hv0 = GRU(onehot(t), 0), msg0 = gated msg
    gi = w_ih + b_ih[:, None]                       # [1503, 8]
    r0 = _sigmoid(gi[0:501] + b_hh[0:501, None])
    z0 = _sigmoid(gi[501:1002] + b_hh[501:1002, None])
    n0 = np.tanh(gi[1002:1503] + r0 * b_hh[1002:1503, None])
    hv0 = (1.0 - z0) * n0                           # [501, 8]
    g0 = _sigmoid(gate_w[:, :HS] @ hv0 + (gate_b + gate_w[:, HS])[:, None])
    m0 = map_w[:, :HS] @ hv0 + map_w[:, HS][:, None]
    msg0 = (g0 * m0).astype(f32)                    # [501, 8]

    fcb = np.zeros((128, 1), f32)
    fcb[0:56, 0] = fc1_b
    fcb[56:112, 0] = fc2_b
    ident = np.eye(128, dtype=f32).astype(bf16)
    return dict(wa=ktile_flat(WA, 1024), wc=ktile_flat(WC, 512),
                wgm=wgm.astype(bf16), w3s=w3s.astype(bf16),
                w3r=w3r.astype(bf16),
                wf=ktile_flat(WF, 112), fcb=fcb, ident=ident), msg0


def _prep_core(node_types, adj, w_ih, b_ih, msg0, core):
    import ml_dtypes
    f32 = np.float32
    bf16 = ml_dtypes.bfloat16
    off = core * BL
    nt = np.asarray(node_types[off:off + BL])       # [256, 16] int32
    ad = np.asarray(adj[off:off + BL], dtype=f32)   # [256, 16, 16]

    # x (one-hot + const-1), batch-major [128, 16*2*9]
    xh = np.zeros((128, 16 * NBT * 9), f32)
    for v in range(16):
        for bt in range(NBT):
            nb = nt[bt * 128:(bt + 1) * 128, v]
            base = (v * NBT + bt) * 9
            xh[:, base:base + 8] = (
                nb[:, None] == np.arange(NVT)[None, :]).astype(f32)
            xh[:, base + 8] = 1.0

    # step-0 message per graph + initial hT for step 1
    msg0_g = msg0[:, nt[:, 0]].T                    # [256, 501]
    ms0 = np.zeros((128, NBT * 512), f32)
    for bt in range(NBT):
        ms0[:, bt * 512:bt * 512 + 501] = msg0_g[bt * 128:(bt + 1) * 128]
    h_in1 = ad[:, 1, 0:1] * msg0_g                  # [256, 501]
    HT1 = np.zeros((512, 256), f32)
    HT1[0:501] = h_in1.T
    HT1[501:509] = (nt[:, 1][:, None] == np.arange(NVT)[None, :]).T
    HT1[509] = 1.0
    ht1 = np.ascontiguousarray(
        HT1.reshape(4, 128, 256).transpose(1, 0, 2).reshape(128, 1024))

    # Bn = w_ih_n @ x + b_ih_n, feature-major ktiles [128, 16*1024]
    W3 = np.asarray(w_ih[1002:1503], f32)           # [501, 8]
    B3 = np.asarray(b_ih[1002:1503], f32)
    bnf = np.zeros((128, 16 * 1024), f32)
    for v in range(1, 16):
        BN = np.zeros((512, 256), f32)
        BN[0:501] = W3[:, nt[:, v]] + B3[:, None]
        bnf[:, v * 1024:(v + 1) * 1024] = (
            BN.reshape(4, 128, 256).transpose(1, 0, 2).reshape(128, 1024))

    # diagonal mask tiles per bt: [128, 120*128] (vn >= 2 used)
    dm = []
    ar = np.arange(128)
    for bt in range(NBT):
        blocks = np.zeros((128, 120, 128), f32)
        for vn in range(2, 16):
            for u in range(vn):
                blocks[ar, _tri(vn) + u, ar] = ad[bt * 128 + ar, vn, u]
        dm.append(np.ascontiguousarray(
            blocks.reshape(128, 120 * 128)).astype(bf16))

    return dict(xh=xh.astype(bf16), bnf=bnf.astype(bf16),
                ht1=ht1.astype(bf16), ms0=ms0.astype(bf16),
                dmf0=dm[0], dmf1=dm[1])


def _prep_all(inputs):
    static, msg0 = _prep_static(
        np.asarray(inputs["w_ih"], np.float32),
        np.asarray(inputs["w_hh"], np.float32),
        np.asarray(inputs["b_ih"], np.float32),
        np.asarray(inputs["b_hh"], np.float32),
        np.asarray(inputs["gate_w"], np.float32),
        np.asarray(inputs["gate_b"], np.float32),
        np.asarray(inputs["map_w"], np.float32),
        np.asarray(inputs["fc1_w"], np.float32),
        np.asarray(inputs["fc1_b"], np.float32),
        np.asarray(inputs["fc2_w"], np.float32),
        np.asarray(inputs["fc2_b"], np.float32))
    in_maps = []
    for c in range(NC_CORES):
        m = dict(static)
        m.update(_prep_core(inputs["node_types"], inputs["adj"],
                            np.asarray(inputs["w_ih"], np.float32),
                            np.asarray(inputs["b_ih"], np.float32),
                            msg0, c))
        in_maps.append(m)
    return in_maps


def kernel(node_types, adj, w_ih, w_hh, b_ih, b_hh, gate_w, gate_b, map_w,
           fc1_w, fc1_b, fc2_w, fc2_b):
    from concourse.bass_utils import run_bass_kernel_spmd

    if "nc" not in _CACHE:
        _CACHE["nc"] = _build_nc()
    nc = _CACHE["nc"]

    in_maps = _prep_all(dict(
        node_types=node_types, adj=adj, w_ih=w_ih, w_hh=w_hh, b_ih=b_ih,
        b_hh=b_hh, gate_w=gate_w, gate_b=gate_b, map_w=map_w,
        fc1_w=fc1_w, fc1_b=fc1_b, fc2_w=fc2_w, fc2_b=fc2_b))

    res = run_bass_kernel_spmd(nc, in_maps, core_ids=list(range(NC_CORES)))
    ys = [res.results[c]["y"] for c in range(NC_CORES)]   # each [112, 256]
    out = np.concatenate(ys, axis=1).T                     # [2048, 112]
    return np.ascontiguousarray(out.astype(np.float32))
